# revision 1
# baseline (speedup 1.0000x reference)
"""Trainium2 Bass kernel for nn_Attention_77043123355775.

Sharded GQA causal attention with RoPE: 8 NeuronCores as 2-way data
parallel (batch) x 4-way tensor parallel (heads). Each core computes its
4 Q heads / 2 KV heads for one batch entry and a partial output
projection (x[b] @ W)^T; the host sums the 4 partials per batch.

All matmuls use bf16 hi/lo splitting (3 matmuls per logical fp32 matmul),
giving ~1e-5 relative error at ~3x bf16 matmul cost. Scores are computed
transposed (k on partitions) so the kernel needs no on-chip transposes.
"""
import math
import os
import sys

for _p in ("/opt/trn_rl_repo",):
    if _p not in sys.path:
        sys.path.insert(0, _p)

import ml_dtypes
import numpy as np

import concourse.bass as bass
import concourse.mybir as mybir
import concourse.tile as tile

from concourse.tile import add_dep_helper

dt = mybir.dt
AF = mybir.ActivationFunctionType


def build_attention_nc(S=2048, D=2048, NQ=4, NKV=2, HD=128, TC=512):
    assert HD == 128
    C = D // 128          # contraction chunks over features
    TB = S // 128         # 128-token blocks
    NTC = S // TC         # token chunks
    DB = D // 128         # output feature blocks
    CO = NQ * HD // 128   # contraction chunks for wo (= NQ)
    REP = NQ // NKV
    CH = C // 2           # c-chunks per x half-tile
    scale = 1.0 / math.sqrt(HD)

    nc = bass.Bass()

    xh = nc.dram_tensor("xh", [D, S], dt.bfloat16, kind="ExternalInput")
    xl = nc.dram_tensor("xl", [D, S], dt.bfloat16, kind="ExternalInput")
    wqp = nc.dram_tensor("wqp", [D, NQ * 2 * HD], dt.bfloat16, kind="ExternalInput")
    wkp = nc.dram_tensor("wkp", [D, NKV * 2 * HD], dt.bfloat16, kind="ExternalInput")
    wvp = nc.dram_tensor("wvp", [D, NKV * 2 * HD], dt.bfloat16, kind="ExternalInput")
    woh = nc.dram_tensor("woh", [NQ * HD, D], dt.bfloat16, kind="ExternalInput")
    wol = nc.dram_tensor("wol", [NQ * HD, D], dt.bfloat16, kind="ExternalInput")
    csT = nc.dram_tensor("csT", [HD, S], dt.float32, kind="ExternalInput")
    masks = nc.dram_tensor("masks", [4 * 128, TC], dt.bfloat16, kind="ExternalInput")
    outT = nc.dram_tensor("outT", [D, S], dt.float32, kind="ExternalOutput")

    with tile.TileContext(nc) as tc:
        with (
            tc.tile_pool(name="const", bufs=1) as constp,
            tc.tile_pool(name="tabs", bufs=1) as tabp,
            tc.tile_pool(name="acts", bufs=1) as actp,
            tc.tile_pool(name="chunkacts", bufs=1) as cap,
            tc.tile_pool(name="wstream", bufs=2) as wsp,
            tc.tile_pool(name="xstream", bufs=2) as xsp,
            tc.tile_pool(name="wo", bufs=1) as wop,
            tc.tile_pool(name="scratch", bufs=3) as scr,
            tc.tile_pool(name="psum", bufs=1, space="PSUM") as psp,
        ):
            ones_t = constp.tile([128, 1], dt.bfloat16, tag="ones")
            nc.vector.memset(ones_t[:], 1.0)
            ones_row = constp.tile([1, 128], dt.float32, tag="ones_row")
            nc.vector.memset(ones_row[:], 1.0)

            cs_t = tabp.tile([HD, S], dt.float32, tag="cs")
            nc.gpsimd.dma_start(cs_t[:], csT[:])
            cos_t = cs_t[0:HD // 2, :]
            sin_t = cs_t[HD // 2:HD, :]
            mask_t = [tabp.tile([128, TC], dt.bfloat16, tag=f"mask{i}", name=f"mask{i}") for i in range(4)]

            # wo resident; loaded on the gpsimd ring as 2 big DMAs, deferred
            # into the chunk-0 attention section
            woh_t = wop.tile([128, CO * D], dt.bfloat16, tag="woh")
            wol_t = wop.tile([128, CO * D], dt.bfloat16, tag="wol")

            # K/V persist for the full sequence (written chunk by chunk)
            kth = [actp.tile([128, S], dt.bfloat16, tag=f"kth{h}", name=f"kth{h}") for h in range(NKV)]
            ktl = [actp.tile([128, S], dt.bfloat16, tag=f"ktl{h}", name=f"ktl{h}") for h in range(NKV)]
            vh_t = [actp.tile([128, NKV * HD], dt.bfloat16, tag=f"vh{b}", name=f"vh{b}") for b in range(TB)]
            vl_t = [actp.tile([128, NKV * HD], dt.bfloat16, tag=f"vl{b}", name=f"vl{b}") for b in range(TB)]

            for tci in range(NTC):
                ts = slice(tci * TC, (tci + 1) * TC)
                qth = [cap.tile([128, TC], dt.bfloat16, tag=f"qth{h}", name=f"qth{h}_{tci}") for h in range(NQ)]
                qtl = [cap.tile([128, TC], dt.bfloat16, tag=f"qtl{h}", name=f"qtl{h}_{tci}") for h in range(NQ)]
                oth = [cap.tile([128, TC], dt.bfloat16, tag=f"oth{h}", name=f"oth{h}_{tci}") for h in range(NQ)]
                otl = [cap.tile([128, TC], dt.bfloat16, tag=f"otl{h}", name=f"otl{h}_{tci}") for h in range(NQ)]

                # ---- big-DMA input streams for chunk tci ----
                # wqk head 0 first on the sync ring so PE can start ASAP
                wqk_t = []
                for h in range(NQ + NKV):
                    whl = wsp.tile([128, C * 2 * HD], dt.bfloat16, tag="wqk_s",
                                   bufs=3, name=f"wqk_{tci}_{h}")
                    wqk_t.append(whl)
                wsrcs = [wqp] * NQ + [wkp] * NKV
                wcols = [h * 2 * HD for h in range(NQ)] + [h * 2 * HD for h in range(NKV)]

                wqk_dma = {}

                def dma_wqk(h):
                    src = wsrcs[h][:, wcols[h]:wcols[h] + 2 * HD]
                    wqk_dma[h] = nc.sync.dma_start(
                        wqk_t[h].rearrange("p (c n) -> p c n", c=C),
                        src.rearrange("(c p) n -> p c n", p=128),
                    )

                # Ring plan per chunk (pseudo-DMAs block their issuing
                # engine for the whole transfer): sync carries wqk + x-hi,
                # scalar carries x-lo + wv, gpsimd carries stores + consts.
                # x is loaded in quarters interleaved with wqk so PE starts
                # on the first quarter instead of waiting for half/full x.
                CQ = max(C // 4, 1)
                NG = C // CQ
                xh_g = []
                xl_g = []
                for g in range(NG):
                    rs = slice(g * CQ * 128, (g + 1) * CQ * 128)
                    th = xsp.tile([128, CQ * TC], dt.bfloat16, tag="xh", bufs=NG, name=f"xh_{tci}_{g}")
                    tl = xsp.tile([128, CQ * TC], dt.bfloat16, tag="xl", bufs=NG, name=f"xl_{tci}_{g}")
                    if g == 0:
                        dma_wqk(0)
                    nc.sync.dma_start(
                        th.rearrange("p (c n) -> p c n", c=CQ),
                        xh[rs, ts].rearrange("(c p) n -> p c n", p=128),
                    )
                    nc.scalar.dma_start(
                        tl.rearrange("p (c n) -> p c n", c=CQ),
                        xl[rs, ts].rearrange("(c p) n -> p c n", p=128),
                    )
                    if 1 + g < NQ + NKV:
                        dma_wqk(1 + g)
                    xh_g.append(th)
                    xl_g.append(tl)
                for h in range(NG + 1, NQ + NKV):
                    dma_wqk(h)
                CW = CQ
                # wv halves: gpsimd ring at chunk 0 (keeps the startup
                # burst off the HWDGE rings), scalar ring afterwards
                wv_ring = nc.gpsimd if tci == 0 else nc.scalar
                wv_g = []
                for g in range(2):
                    rs = slice(g * CH * 128, (g + 1) * CH * 128)
                    t = wsp.tile([128, CH * 2 * NKV * HD], dt.bfloat16, tag="wv_s",
                                 bufs=2, name=f"wv_{tci}_{g}")
                    wv_ring.dma_start(
                        t.rearrange("p (c n) -> p c n", c=CH),
                        wvp[rs, :].rearrange("(c p) n -> p c n", p=128),
                    )
                    wv_g.append(t)
                if tci == 0:
                    for i in range(4):
                        nc.gpsimd.dma_start(mask_t[i][:], masks[i * 128:(i + 1) * 128, :])

                def xh_c(c):
                    return xh_g[c // CW][:, (c % CW) * TC:(c % CW + 1) * TC]

                def xl_c(c):
                    return xl_g[c // CW][:, (c % CW) * TC:(c % CW + 1) * TC]

                # ---- QKV projections + RoPE + split ----
                first_mm = {}
                for h in range(NQ + NKV):
                    is_q = h < NQ
                    ps = psp.tile([128, TC], dt.float32, tag="mm", bufs=2)
                    n_mm = 3 * C
                    i_mm = 0
                    for c in range(C):
                        wht = wqk_t[h][:, c * 2 * HD:c * 2 * HD + HD]
                        wlt = wqk_t[h][:, c * 2 * HD + HD:(c + 1) * 2 * HD]
                        for lhsT, rhs in ((wht, xh_c(c)), (wht, xl_c(c)), (wlt, xh_c(c))):
                            mm = nc.tensor.matmul(
                                ps[:], lhsT, rhs,
                                start=(i_mm == 0), stop=(i_mm == n_mm - 1),
                            )
                            if i_mm == 0:
                                first_mm[h] = mm
                            i_mm += 1
                    # chunk 0: pace the weight prefetch two heads ahead so
                    # the startup HBM burst stays small (all 8 cores share
                    # chip bandwidth at packet granularity)
                    if tci == 0 and h + 2 in wqk_dma:
                        add_dep_helper(wqk_dma[h + 2].ins, first_mm[h].ins,
                                       reason="startup prefetch throttle")
                    # RoPE in f32 from PSUM -> scratch
                    rot = scr.tile([128, TC], dt.float32, tag="rope", bufs=2)
                    t0 = scr.tile([128, TC], dt.float32, tag="ropetmp", bufs=1)
                    cs = cos_t[:, ts]
                    sn = sin_t[:, ts]
                    xr = ps[0:64, :]
                    xi = ps[64:128, :]
                    nc.vector.tensor_tensor(rot[0:64, :], xr, cs, mybir.AluOpType.mult)
                    nc.vector.tensor_tensor(t0[0:64, :], xi, sn, mybir.AluOpType.mult)
                    nc.vector.tensor_tensor(rot[0:64, :], rot[0:64, :], t0[0:64, :], mybir.AluOpType.subtract)
                    nc.vector.tensor_tensor(rot[64:128, :], xr, sn, mybir.AluOpType.mult)
                    nc.vector.tensor_tensor(t0[64:128, :], xi, cs, mybir.AluOpType.mult)
                    nc.vector.tensor_tensor(rot[64:128, :], rot[64:128, :], t0[64:128, :], mybir.AluOpType.add)
                    if is_q:
                        dsth, dstl = qth[h][:], qtl[h][:]
                    else:
                        dsth, dstl = kth[h - NQ][:, ts], ktl[h - NQ][:, ts]
                    nc.vector.tensor_copy(dsth, rot[:])
                    nc.vector.tensor_tensor(dstl, rot[:], dsth, mybir.AluOpType.subtract)

                # V projection
                for tb in range(TC // 128):
                    tbg = tci * (TC // 128) + tb
                    ps = psp.tile([128, NKV * HD], dt.float32, tag="mm", bufs=2)
                    n_mm = 3 * C
                    i_mm = 0
                    for c in range(C):
                        xh_s = xh_c(c)[:, tb * 128:(tb + 1) * 128]
                        xl_s = xl_c(c)[:, tb * 128:(tb + 1) * 128]
                        g, cc = c // CH, c % CH
                        vht = wv_g[g][:, cc * 2 * NKV * HD:cc * 2 * NKV * HD + NKV * HD]
                        vlt = wv_g[g][:, cc * 2 * NKV * HD + NKV * HD:(cc + 1) * 2 * NKV * HD]
                        for lhsT, rhs in ((xh_s, vht), (xh_s, vlt), (xl_s, vht)):
                            nc.tensor.matmul(
                                ps[:], lhsT, rhs,
                                start=(i_mm == 0), stop=(i_mm == n_mm - 1),
                            )
                            i_mm += 1
                    nc.vector.tensor_copy(vh_t[tbg][:], ps[:])
                    nc.vector.tensor_tensor(vl_t[tbg][:], ps[:], vh_t[tbg][:], mybir.AluOpType.subtract)

                # ---- attention for q-chunk tci (keys 0..(tci+1)*TC) ----
                if tci == 0:
                    nc.gpsimd.dma_start(
                        woh_t.rearrange("p (c n) -> p c n", c=CO),
                        woh.rearrange("(c p) n -> p c n", p=128),
                    )
                    nc.gpsimd.dma_start(
                        wol_t.rearrange("p (c n) -> p c n", c=CO),
                        wol.rearrange("(c p) n -> p c n", p=128),
                    )
                qc = tci
                nkb = (qc + 1) * (TC // 128)
                pending_norm = []

                def emit_norm(h, ot_ps, sum_ps):
                    rec = scr.tile([1, TC], dt.float32, tag="rec", bufs=1, name=f"rec_{tci}_{h}")
                    nc.vector.reciprocal(rec[:], sum_ps[:])
                    bc_ps = psp.tile([128, TC], dt.float32, tag="bcast", bufs=1, name=f"bc_{tci}_{h}")
                    nc.tensor.matmul(bc_ps[:], ones_row[:], rec[:], start=True, stop=True)
                    recb = scr.tile([128, TC], dt.float32, tag="recb", bufs=1, name=f"recb_{tci}_{h}")
                    nc.scalar.copy(recb[:], bc_ps[:])
                    otn = scr.tile([128, TC], dt.float32, tag="otn", bufs=2, name=f"otn_{tci}_{h}")
                    nc.vector.tensor_tensor(otn[:], ot_ps[:], recb[:], mybir.AluOpType.mult)
                    nc.vector.tensor_copy(oth[h][:], otn[:])
                    nc.vector.tensor_tensor(otl[h][:], otn[:], oth[h][:], mybir.AluOpType.subtract)

                # Two-stage software pipeline over all (head, block)
                # pairs: scores/exp/split lead PV by LAG blocks so the PE
                # never waits on the ACT/DVE probs chain at head starts.
                LAG = 4
                blocks = [(h, kb) for h in range(NQ) for kb in range(nkb)]
                head_ps = {}
                head_sum = {}

                def emit_scores(h, kb):
                    kv = h // REP
                    d = kb * 128 - qc * TC
                    ks = slice(kb * 128, (kb + 1) * 128)
                    q0 = max(d, 0)
                    sc_ps = psp.tile([128, TC], dt.float32, tag="mm", bufs=2,
                                     name=f"sc_{tci}_{h}_{kb}")
                    mms = (
                        (kth[kv][:, ks], qth[h][:, q0:TC]),
                        (kth[kv][:, ks], qtl[h][:, q0:TC]),
                        (ktl[kv][:, ks], qth[h][:, q0:TC]),
                    )
                    for i_mm, (lhsT, rhs) in enumerate(mms):
                        nc.tensor.matmul(sc_ps[:, q0:TC], lhsT, rhs, start=(i_mm == 0), stop=(i_mm == 2))
                    pf = scr.tile([128, TC], dt.float32, tag="pf", bufs=3,
                                  name=f"pf_{tci}_{h}_{kb}")
                    nc.scalar.activation(pf[:, q0:TC], sc_ps[:, q0:TC], AF.Exp, bias=0.0, scale=scale)
                    if d >= 0:
                        nc.vector.tensor_tensor(pf[:, q0:TC], pf[:, q0:TC], mask_t[d // 128][:, q0:TC], mybir.AluOpType.mult)
                    ph = scr.tile([128, TC], dt.bfloat16, tag="ph", bufs=LAG + 2,
                                  name=f"ph_{tci}_{h}_{kb}")
                    pl = scr.tile([128, TC], dt.bfloat16, tag="pl", bufs=LAG + 2,
                                  name=f"pl_{tci}_{h}_{kb}")
                    nc.scalar.copy(ph[:, q0:TC], pf[:, q0:TC])
                    nc.vector.tensor_tensor(pl[:, q0:TC], pf[:, q0:TC], ph[:, q0:TC], mybir.AluOpType.subtract)
                    return ph, pl

                def emit_pv(h, kb, ph, pl):
                    kv = h // REP
                    vcol = kv * HD
                    d = kb * 128 - qc * TC
                    q0 = max(d, 0)
                    if kb == 0:
                        head_ps[h] = (
                            psp.tile([128, TC], dt.float32, tag="otps", bufs=3,
                                     name=f"ot_{tci}_{h}"),
                            psp.tile([1, TC], dt.float32, tag="sums", bufs=2,
                                     name=f"sum_{tci}_{h}"),
                        )
                    ot_ps, sum_ps = head_ps[h]
                    vh_s = vh_t[kb][:, vcol:vcol + HD]
                    vl_s = vl_t[kb][:, vcol:vcol + HD]
                    pv = ((vh_s, ph[:, q0:TC]), (vh_s, pl[:, q0:TC]), (vl_s, ph[:, q0:TC]))
                    for i_mm, (lhsT, rhs) in enumerate(pv):
                        nc.tensor.matmul(
                            ot_ps[:, q0:TC], lhsT, rhs,
                            start=(kb == 0 and i_mm == 0),
                            stop=(kb == nkb - 1 and i_mm == 2),
                        )
                    for i_mm, rhs in enumerate((ph[:, q0:TC], pl[:, q0:TC])):
                        nc.tensor.matmul(
                            sum_ps[:, q0:TC], ones_t[:], rhs,
                            start=(kb == 0 and i_mm == 0),
                            stop=(kb == nkb - 1 and i_mm == 1),
                        )
                    if kb == nkb - 1:
                        pending_norm.append((h, ot_ps, sum_ps))
                        if len(pending_norm) > 1:
                            emit_norm(*pending_norm.pop(0))

                probs_q = []
                for h, kb in blocks:
                    probs_q.append((h, kb, emit_scores(h, kb)))
                    if len(probs_q) > LAG:
                        hh, kk, (ph, pl) = probs_q.pop(0)
                        emit_pv(hh, kk, ph, pl)
                for hh, kk, (ph, pl) in probs_q:
                    emit_pv(hh, kk, ph, pl)

                # ---- output projection for token-chunk tci ----
                for db in range(DB):
                    ds_ = slice(db * 128, (db + 1) * 128)
                    ps = psp.tile([128, TC], dt.float32, tag="mm", bufs=2)
                    n_mm = 3 * CO
                    i_mm = 0
                    for c in range(CO):
                        # the last head's normalization drains here, covered
                        # by the first db's head-0..2 matmuls
                        if db == 0 and c == CO - 1 and pending_norm:
                            for args in pending_norm:
                                emit_norm(*args)
                            pending_norm = []
                        wh_s = woh_t[:, c * D + db * 128:c * D + (db + 1) * 128]
                        wl_s = wol_t[:, c * D + db * 128:c * D + (db + 1) * 128]
                        for lhsT, rhs in (
                            (wh_s, oth[c][:]),
                            (wh_s, otl[c][:]),
                            (wl_s, oth[c][:]),
                        ):
                            nc.tensor.matmul(
                                ps[:], lhsT, rhs,
                                start=(i_mm == 0), stop=(i_mm == n_mm - 1),
                            )
                            i_mm += 1
                    o3 = scr.tile([128, TC], dt.float32, tag="o3", bufs=2)
                    nc.scalar.copy(o3[:], ps[:])
                    eng = nc.sync if tci == NTC - 1 else nc.gpsimd
                    eng.dma_start(outT[ds_, ts], o3[:])

    return nc


# ---------------------------------------------------------------------------
# walrus in this container refuses >1 sem wait per instruction ("Too many
# sync wait commands"). Hoist excess waits onto same-engine NoOps inserted
# immediately before the instruction - program order on the engine queue
# preserves the sync semantics.
def split_multiwait_insts(nc, max_waits=1):
    n_split = 0
    for bb in nc.main_func.blocks:
        insts = bb.instructions
        i = 0
        while i < len(insts):
            ins = insts[i]
            si = getattr(ins, "sync_info", None)
            if si is not None and si.on_wait and len(si.on_wait) > max_waits:
                waits = list(si.on_wait)
                head, tail = waits[:-max_waits], waits[-max_waits:]
                nops = []
                for j in range(0, len(head), max_waits):
                    nop = mybir.InstNoOp(name=f"{ins.name}-ws{j}", ins=[], outs=[])
                    nop.engine = ins.engine
                    nop.sync_info = mybir.SyncInfo(
                        on_wait=head[j:j + max_waits], on_update=[])
                    nops.append(nop)
                ins.sync_info = mybir.SyncInfo(
                    on_wait=tail, on_update=list(si.on_update or []))
                insts[i:i] = nops
                i += len(nops)
                n_split += 1
            i += 1
    return n_split


# ---------------------------------------------------------------------------
# Host-side shard preparation / gather
BF16 = ml_dtypes.bfloat16


BF16 = ml_dtypes.bfloat16


def _split(a):
    h = a.astype(BF16)
    l = (a.astype(np.float32) - h.astype(np.float32)).astype(BF16)
    return h, l


def rope_tables(S, HD):
    inv = 1.0 / (10000.0 ** (np.arange(0, HD, 2, dtype=np.float32) / HD))
    t = np.arange(S, dtype=np.float32)
    f = np.outer(t, inv).astype(np.float32)  # [S, HD//2]
    return np.ascontiguousarray(np.cos(f).T), np.ascontiguousarray(np.sin(f).T)


def causal_masks(TC):
    # masks[dd][k, qrel] = 1 if k + dd*128 <= qrel else 0
    out = np.zeros((4 * 128, TC), BF16)
    k = np.arange(128)[:, None]
    q = np.arange(TC)[None, :]
    for dd in range(4):
        out[dd * 128:(dd + 1) * 128] = (k + dd * 128 <= q).astype(BF16)
    return out


def rope_perm(HD):
    # new row i (i < HD//2) = old 2i; new row HD//2+i = old 2i+1
    return np.concatenate([np.arange(0, HD, 2), np.arange(1, HD, 2)])


def make_in_maps(x, wq, wk, wv, wo, *, n_batch_shards, n_head_shards,
                 NQ_TOT, NKV_TOT, HD, TC):
    """Returns list of in_maps, one per core (batch-major: core = b*G + g)."""
    B, S, D = x.shape
    G = n_head_shards
    NQ = NQ_TOT // G
    NKV = NKV_TOT // G
    perm = rope_perm(HD)
    cosT, sinT = rope_tables(S, HD)
    csT = np.concatenate([cosT, sinT], axis=0)  # [HD, S]
    masks = causal_masks(TC)

    # Per-batch xT splits (shared across head shards)
    xt = {}
    for b in range(B):
        xT = np.ascontiguousarray(x[b].T)  # [D, S]
        xt[b] = _split(xT)

    def _pack_per_head(wT_h, wT_l, n_heads):
        # [D, n_heads*HD] hi/lo -> [D, n_heads*2*HD] with per-head [hi | lo]
        D_ = wT_h.shape[0]
        out = np.empty((D_, n_heads * 2 * HD), BF16)
        for hh in range(n_heads):
            out[:, hh * 2 * HD:hh * 2 * HD + HD] = wT_h[:, hh * HD:(hh + 1) * HD]
            out[:, hh * 2 * HD + HD:(hh + 1) * 2 * HD] = wT_l[:, hh * HD:(hh + 1) * HD]
        return out

    # Per-headgroup weight shards
    wshard = {}
    for g in range(G):
        qrows = slice(g * NQ * HD, (g + 1) * NQ * HD)
        kvrows = slice(g * NKV * HD, (g + 1) * NKV * HD)
        wq_g = wq[qrows, :].copy()      # [NQ*HD, D]
        wk_g = wk[kvrows, :].copy()
        wv_g = wv[kvrows, :].copy()
        # RoPE permutation of output rows, per head
        for hh in range(NQ):
            blk = wq_g[hh * HD:(hh + 1) * HD]
            wq_g[hh * HD:(hh + 1) * HD] = blk[perm]
        for hh in range(NKV):
            blk = wk_g[hh * HD:(hh + 1) * HD]
            wk_g[hh * HD:(hh + 1) * HD] = blk[perm]
        wqT = np.ascontiguousarray(wq_g.T)   # [D, NQ*HD]
        wkT = np.ascontiguousarray(wk_g.T)
        wvT = np.ascontiguousarray(wv_g.T)
        woT = np.ascontiguousarray(wo[:, qrows].T)  # [NQ*HD, D]
        wqp = _pack_per_head(*_split(wqT), NQ)
        wkp = _pack_per_head(*_split(wkT), NKV)
        wvh_, wvl_ = _split(wvT)
        wvp = np.concatenate([wvh_, wvl_], axis=1)  # [D, 2*NKV*HD] hi-all|lo-all
        wshard[g] = (wqp, wkp, wvp, _split(woT))

    in_maps = []
    for b in range(n_batch_shards):
        for g in range(G):
            wqp, wkp, wvp, (woh, wol) = wshard[g]
            xh, xl = xt[b]
            in_maps.append({
                "xh": xh, "xl": xl,
                "wqp": wqp, "wkp": wkp, "wvp": wvp,
                "woh": woh, "wol": wol,
                "csT": csT,
                "masks": masks,
            })
    return in_maps


def combine_outputs(outTs, B, G):
    """outTs: list of [D, S] partials, core order b*G+g. Returns [B, S, D]."""
    outs = []
    for b in range(B):
        acc = outTs[b * G].astype(np.float32).copy()
        for g in range(1, G):
            acc += outTs[b * G + g]
        outs.append(acc.T)  # [S, D]
    return np.stack(outs)


_NC_CACHE = {}


def _get_nc(S, D, NQ, NKV, HD, TC):
    key = (S, D, NQ, NKV, HD, TC)
    if key not in _NC_CACHE:
        nc = build_attention_nc(S=S, D=D, NQ=NQ, NKV=NKV, HD=HD, TC=TC)
        split_multiwait_insts(nc)
        _NC_CACHE[key] = nc
    return _NC_CACHE[key]


def kernel(**inputs):
    x = np.asarray(inputs["x"], dtype=np.float32)
    wq = np.asarray(inputs["wq"], dtype=np.float32)
    wk = np.asarray(inputs["wk"], dtype=np.float32)
    wv = np.asarray(inputs["wv"], dtype=np.float32)
    wo = np.asarray(inputs["wo"], dtype=np.float32)

    B, S, D = x.shape          # (2, 2048, 2048)
    NQ_TOT = wq.shape[0] // 128
    NKV_TOT = wk.shape[0] // 128
    HD = 128
    TC = 512
    G = 4                      # head shards
    NQ, NKV = NQ_TOT // G, NKV_TOT // G

    nc = _get_nc(S, D, NQ, NKV, HD, TC)
    in_maps = make_in_maps(
        x, wq, wk, wv, wo,
        n_batch_shards=B, n_head_shards=G,
        NQ_TOT=NQ_TOT, NKV_TOT=NKV_TOT, HD=HD, TC=TC,
    )

    from concourse.bass_utils import run_bass_kernel_spmd

    trace = os.environ.get("BASS_ATTN_TRACE") == "1"
    res = run_bass_kernel_spmd(nc, in_maps, list(range(len(in_maps))), trace=trace)
    kernel.last_results = res
    outTs = [r["outT"] for r in res.results]
    return combine_outputs(outTs, B, G).astype(np.float32)



# revision 3
# speedup vs baseline: 2.5769x; 2.5769x over previous
"""Trainium2 Bass kernel for nn_Attention_77043123355775.

Sharded GQA causal attention with RoPE: 8 NeuronCores as 2-way data
parallel (batch) x 4-way tensor parallel (heads). Each core computes its
4 Q heads / 2 KV heads for one batch entry and a partial output
projection (x[b] @ W)^T; the host sums the 4 partials per batch.

All matmuls are single bf16 (inputs rounded to bf16, fp32 PSUM
accumulation), good for ~1e-3 relative error against the 2e-2 gate at
1/3 the tensor-engine cost of hi/lo splitting. Scores are computed
transposed (k on partitions) so the kernel needs no on-chip transposes.
Weights stay resident in SBUF across all token chunks.
"""
import math
import os
import sys

for _p in ("/opt/trn_rl_repo",):
    if _p not in sys.path:
        sys.path.insert(0, _p)

import ml_dtypes
import numpy as np

import concourse.bass as bass
import concourse.mybir as mybir
import concourse.tile as tile

dt = mybir.dt
AF = mybir.ActivationFunctionType


def build_attention_nc(S=2048, D=2048, NQ=4, NKV=2, HD=128, TC=512):
    assert HD == 128
    C = D // 128          # contraction chunks over features
    TB = S // 128         # 128-token blocks
    NTC = S // TC         # token chunks
    DB = D // 128         # output feature blocks
    CO = NQ * HD // 128   # contraction chunks for wo (= NQ)
    REP = NQ // NKV
    CH = C // 2           # c-chunks per wv half-tile
    CQ = max(C // 4, 1)   # c-chunks per x quarter-tile
    NG = C // CQ
    scale = 1.0 / math.sqrt(HD)

    nc = bass.Bass()

    xb = nc.dram_tensor("xb", [D, S], dt.bfloat16, kind="ExternalInput")
    wqp = nc.dram_tensor("wqp", [D, NQ * HD], dt.bfloat16, kind="ExternalInput")
    wkp = nc.dram_tensor("wkp", [D, NKV * HD], dt.bfloat16, kind="ExternalInput")
    wvp = nc.dram_tensor("wvp", [D, NKV * HD], dt.bfloat16, kind="ExternalInput")
    wot = nc.dram_tensor("wot", [NQ * HD, D], dt.bfloat16, kind="ExternalInput")
    csT = nc.dram_tensor("csT", [HD, S], dt.float32, kind="ExternalInput")
    masks = nc.dram_tensor("masks", [4 * 128, TC], dt.bfloat16, kind="ExternalInput")
    outT = nc.dram_tensor("outT", [D, S], dt.float32, kind="ExternalOutput")

    with tile.TileContext(nc) as tc:
        with (
            tc.tile_pool(name="const", bufs=1) as constp,
            tc.tile_pool(name="tabs", bufs=1) as tabp,
            tc.tile_pool(name="weights", bufs=1) as wp,
            tc.tile_pool(name="acts", bufs=1) as actp,
            tc.tile_pool(name="chunkacts", bufs=1) as cap,
            tc.tile_pool(name="xstream", bufs=2) as xsp,
            tc.tile_pool(name="scratch", bufs=3) as scr,
            tc.tile_pool(name="psum", bufs=1, space="PSUM") as psp,
        ):
            ones_t = constp.tile([128, 1], dt.bfloat16, tag="ones")
            nc.vector.memset(ones_t[:], 1.0)
            ones_row = constp.tile([1, 128], dt.bfloat16, tag="ones_row")
            nc.vector.memset(ones_row[:], 1.0)

            # ---- one-time loads: tables + weights (resident all chunks) ----
            # gpsimd ring: cos/sin, masks, wv, wo; scalar ring: wq, wk.
            cs_t = tabp.tile([HD, S], dt.float32, tag="cs")
            nc.gpsimd.dma_start(cs_t[:], csT[:])
            cos_t = cs_t[0:HD // 2, :]
            sin_t = cs_t[HD // 2:HD, :]
            mask_t = [tabp.tile([128, TC], dt.bfloat16, tag=f"mask{i}", name=f"mask{i}") for i in range(4)]
            for i in range(4):
                nc.gpsimd.dma_start(mask_t[i][:], masks[i * 128:(i + 1) * 128, :])

            wq_t = []
            for h in range(NQ):
                t = wp.tile([128, C * HD], dt.bfloat16, tag=f"wq{h}", name=f"wq{h}")
                nc.scalar.dma_start(
                    t.rearrange("p (c n) -> p c n", c=C),
                    wqp[:, h * HD:(h + 1) * HD].rearrange("(c p) n -> p c n", p=128),
                )
                wq_t.append(t)
            wk_t = []
            for h in range(NKV):
                t = wp.tile([128, C * HD], dt.bfloat16, tag=f"wk{h}", name=f"wk{h}")
                nc.scalar.dma_start(
                    t.rearrange("p (c n) -> p c n", c=C),
                    wkp[:, h * HD:(h + 1) * HD].rearrange("(c p) n -> p c n", p=128),
                )
                wk_t.append(t)
            wv_g = []
            for g in range(2):
                rs = slice(g * CH * 128, (g + 1) * CH * 128)
                t = wp.tile([128, CH * NKV * HD], dt.bfloat16, tag=f"wv{g}", name=f"wv{g}")
                nc.gpsimd.dma_start(
                    t.rearrange("p (c n) -> p c n", c=CH),
                    wvp[rs, :].rearrange("(c p) n -> p c n", p=128),
                )
                wv_g.append(t)
            wo_t = wp.tile([128, CO * D], dt.bfloat16, tag="wo")
            for g in range(2):
                nc.gpsimd.dma_start(
                    wo_t.rearrange("p (c n) -> p c n", c=CO)[:, g * (CO // 2):(g + 1) * (CO // 2), :],
                    wot[g * (CO // 2) * 128:(g + 1) * (CO // 2) * 128, :].rearrange("(c p) n -> p c n", p=128),
                )

            # K persists for the full sequence (written chunk by chunk);
            # V persists per 128-token block
            kth = [actp.tile([128, S], dt.bfloat16, tag=f"kth{h}", name=f"kth{h}") for h in range(NKV)]
            vh_t = [actp.tile([128, NKV * HD], dt.bfloat16, tag=f"vh{b}", name=f"vh{b}") for b in range(TB)]

            # x chunk quarters, double buffered across chunks (prefetch)
            x_tiles = {}

            def emit_x_load(tci):
                ts_ = slice(tci * TC, (tci + 1) * TC)
                g_tiles = []
                for g in range(NG):
                    rs = slice(g * CQ * 128, (g + 1) * CQ * 128)
                    t = xsp.tile([128, CQ * TC], dt.bfloat16, tag="xh",
                                 bufs=2 * NG, name=f"xh_{tci}_{g}")
                    nc.sync.dma_start(
                        t.rearrange("p (c n) -> p c n", c=CQ),
                        xb[rs, ts_].rearrange("(c p) n -> p c n", p=128),
                    )
                    g_tiles.append(t)
                x_tiles[tci] = g_tiles

            emit_x_load(0)

            for tci in range(NTC):
                ts = slice(tci * TC, (tci + 1) * TC)
                xh_g = x_tiles.pop(tci)
                qth = [cap.tile([128, TC], dt.bfloat16, tag=f"qth{h}", name=f"qth{h}_{tci}") for h in range(NQ)]
                oth = [cap.tile([128, TC], dt.bfloat16, tag=f"oth{h}", name=f"oth{h}_{tci}") for h in range(NQ)]

                def xh_c(c):
                    return xh_g[c // CQ][:, (c % CQ) * TC:(c % CQ + 1) * TC]

                # ---- QKV projections + RoPE ----
                for h in range(NQ + NKV):
                    is_q = h < NQ
                    wt = wq_t[h] if is_q else wk_t[h - NQ]
                    ps = psp.tile([128, TC], dt.float32, tag="mm", bufs=2)
                    for c in range(C):
                        nc.tensor.matmul(
                            ps[:], wt[:, c * HD:(c + 1) * HD], xh_c(c),
                            start=(c == 0), stop=(c == C - 1),
                        )
                    # RoPE in f32 from PSUM; DVE does the 4 products,
                    # gpsimd combines into the bf16 destination
                    cs = cos_t[:, ts]
                    sn = sin_t[:, ts]
                    xr = ps[0:64, :]
                    xi = ps[64:128, :]
                    # products placed so each combine's operands share base
                    # partitions (SB+SB ops require equal base partition)
                    ta = scr.tile([128, TC], dt.float32, tag="ropetmp", bufs=2)
                    tb = scr.tile([128, TC], dt.float32, tag="ropetmp2", bufs=2)
                    nc.vector.tensor_tensor(ta[0:64, :], xr, cs, mybir.AluOpType.mult)
                    nc.vector.tensor_tensor(tb[0:64, :], xi, sn, mybir.AluOpType.mult)
                    nc.vector.tensor_tensor(ta[64:128, :], xr, sn, mybir.AluOpType.mult)
                    nc.vector.tensor_tensor(tb[64:128, :], xi, cs, mybir.AluOpType.mult)
                    dsth = qth[h][:] if is_q else kth[h - NQ][:, ts]
                    nc.gpsimd.tensor_tensor(dsth[0:64, :], ta[0:64, :], tb[0:64, :], mybir.AluOpType.subtract)
                    nc.gpsimd.tensor_tensor(dsth[64:128, :], ta[64:128, :], tb[64:128, :], mybir.AluOpType.add)

                # ---- V projection ----
                for tb_i in range(TC // 128):
                    tbg = tci * (TC // 128) + tb_i
                    ps = psp.tile([128, NKV * HD], dt.float32, tag="mm", bufs=2)
                    for c in range(C):
                        xh_s = xh_c(c)[:, tb_i * 128:(tb_i + 1) * 128]
                        g, cc = c // CH, c % CH
                        nc.tensor.matmul(
                            ps[:], xh_s, wv_g[g][:, cc * NKV * HD:(cc + 1) * NKV * HD],
                            start=(c == 0), stop=(c == C - 1),
                        )
                    nc.vector.tensor_copy(vh_t[tbg][:], ps[:])

                # prefetch next chunk's x while attention runs
                if tci + 1 < NTC:
                    emit_x_load(tci + 1)

                # ---- attention for q-chunk tci (keys 0..(tci+1)*TC) ----
                qc = tci
                nkb = (qc + 1) * (TC // 128)
                pending_norm = []

                def emit_norm(h, ot_ps, sum_ps):
                    rec = scr.tile([1, TC], dt.bfloat16, tag="rec", bufs=2, name=f"rec_{tci}_{h}")
                    with nc.allow_low_precision("rel-err gate is 2e-2; bf16 1/sum is plenty"):
                        nc.vector.reciprocal(rec[:], sum_ps[:])
                    bc_ps = psp.tile([128, TC], dt.float32, tag="bcast", bufs=1, name=f"bc_{tci}_{h}")
                    nc.tensor.matmul(bc_ps[:], ones_row[:], rec[:], start=True, stop=True)
                    recb = scr.tile([128, TC], dt.float32, tag="recb", bufs=2, name=f"recb_{tci}_{h}")
                    nc.scalar.copy(recb[:], bc_ps[:])
                    nc.vector.tensor_tensor(oth[h][:], ot_ps[:], recb[:], mybir.AluOpType.mult)

                # Two-stage software pipeline over all (head, block)
                # pairs: scores/exp lead PV by LAG blocks so the PE never
                # waits on the ACT/DVE probs chain at head starts.
                LAG = 4
                blocks = [(h, kb) for h in range(NQ) for kb in range(nkb)]
                head_ps = {}

                def emit_scores(h, kb):
                    kv = h // REP
                    d = kb * 128 - qc * TC
                    ks = slice(kb * 128, (kb + 1) * 128)
                    q0 = max(d, 0)
                    sc_ps = psp.tile([128, TC], dt.float32, tag="mm", bufs=2,
                                     name=f"sc_{tci}_{h}_{kb}")
                    nc.tensor.matmul(sc_ps[:, q0:TC], kth[kv][:, ks], qth[h][:, q0:TC],
                                     start=True, stop=True)
                    ph = scr.tile([128, TC], dt.bfloat16, tag="ph", bufs=LAG + 2,
                                  name=f"ph_{tci}_{h}_{kb}")
                    nc.scalar.activation(ph[:, q0:TC], sc_ps[:, q0:TC], AF.Exp, bias=0.0, scale=scale)
                    if d >= 0:
                        nc.vector.tensor_tensor(ph[:, q0:TC], ph[:, q0:TC], mask_t[d // 128][:, q0:TC], mybir.AluOpType.mult)
                    return ph

                def emit_pv(h, kb, ph):
                    kv = h // REP
                    vcol = kv * HD
                    d = kb * 128 - qc * TC
                    q0 = max(d, 0)
                    if kb == 0:
                        head_ps[h] = (
                            psp.tile([128, TC], dt.float32, tag="otps", bufs=3,
                                     name=f"ot_{tci}_{h}"),
                            psp.tile([1, TC], dt.float32, tag="sums", bufs=2,
                                     name=f"sum_{tci}_{h}"),
                        )
                    ot_ps, sum_ps = head_ps[h]
                    nc.tensor.matmul(
                        ot_ps[:, q0:TC], vh_t[kb][:, vcol:vcol + HD], ph[:, q0:TC],
                        start=(kb == 0), stop=(kb == nkb - 1),
                    )
                    nc.tensor.matmul(
                        sum_ps[:, q0:TC], ones_t[:], ph[:, q0:TC],
                        start=(kb == 0), stop=(kb == nkb - 1),
                    )
                    if kb == nkb - 1:
                        pending_norm.append((h, ot_ps, sum_ps))
                        if len(pending_norm) > 1:
                            emit_norm(*pending_norm.pop(0))

                probs_q = []
                for h, kb in blocks:
                    probs_q.append((h, kb, emit_scores(h, kb)))
                    if len(probs_q) > LAG:
                        hh, kk, ph = probs_q.pop(0)
                        emit_pv(hh, kk, ph)
                for hh, kk, ph in probs_q:
                    emit_pv(hh, kk, ph)

                # ---- output projection for token-chunk tci ----
                # o3 copies on ACT, stores round-robin on gpsimd/scalar rings
                # (sync stays clear for the next chunk's x prefetch)
                for db in range(DB):
                    ds_ = slice(db * 128, (db + 1) * 128)
                    ps = psp.tile([128, TC], dt.float32, tag="mm", bufs=2)
                    i_mm = 0
                    for c in range(CO):
                        # the last head's normalization drains here, covered
                        # by the first db's head-0..2 matmuls
                        if db == 0 and c == CO - 1 and pending_norm:
                            for args in pending_norm:
                                emit_norm(*args)
                            pending_norm = []
                        nc.tensor.matmul(
                            ps[:], wo_t[:, c * D + db * 128:c * D + (db + 1) * 128], oth[c][:],
                            start=(i_mm == 0), stop=(i_mm == CO - 1),
                        )
                        i_mm += 1
                    o3 = scr.tile([128, TC], dt.float32, tag="o3", bufs=3)
                    nc.scalar.copy(o3[:], ps[:])
                    eng = nc.gpsimd if db % 2 == 0 else nc.scalar
                    eng.dma_start(outT[ds_, ts], o3[:])

    return nc


# ---------------------------------------------------------------------------
# walrus in this container refuses >1 sem wait per instruction ("Too many
# sync wait commands"). Hoist excess waits onto same-engine NoOps inserted
# immediately before the instruction - program order on the engine queue
# preserves the sync semantics.
def split_multiwait_insts(nc, max_waits=1):
    n_split = 0
    for bb in nc.main_func.blocks:
        insts = bb.instructions
        i = 0
        while i < len(insts):
            ins = insts[i]
            si = getattr(ins, "sync_info", None)
            if si is not None and si.on_wait and len(si.on_wait) > max_waits:
                waits = list(si.on_wait)
                head, tail = waits[:-max_waits], waits[-max_waits:]
                nops = []
                for j in range(0, len(head), max_waits):
                    nop = mybir.InstNoOp(name=f"{ins.name}-ws{j}", ins=[], outs=[])
                    nop.engine = ins.engine
                    nop.sync_info = mybir.SyncInfo(
                        on_wait=head[j:j + max_waits], on_update=[])
                    nops.append(nop)
                ins.sync_info = mybir.SyncInfo(
                    on_wait=tail, on_update=list(si.on_update or []))
                insts[i:i] = nops
                i += len(nops)
                n_split += 1
            i += 1
    return n_split


# ---------------------------------------------------------------------------
# Host-side shard preparation / gather
BF16 = ml_dtypes.bfloat16


def rope_tables(S, HD):
    inv = 1.0 / (10000.0 ** (np.arange(0, HD, 2, dtype=np.float32) / HD))
    t = np.arange(S, dtype=np.float32)
    f = np.outer(t, inv).astype(np.float32)  # [S, HD//2]
    return np.ascontiguousarray(np.cos(f).T), np.ascontiguousarray(np.sin(f).T)


def causal_masks(TC):
    # masks[dd][k, qrel] = 1 if k + dd*128 <= qrel else 0
    out = np.zeros((4 * 128, TC), BF16)
    k = np.arange(128)[:, None]
    q = np.arange(TC)[None, :]
    for dd in range(4):
        out[dd * 128:(dd + 1) * 128] = (k + dd * 128 <= q).astype(BF16)
    return out


def rope_perm(HD):
    # new row i (i < HD//2) = old 2i; new row HD//2+i = old 2i+1
    return np.concatenate([np.arange(0, HD, 2), np.arange(1, HD, 2)])


def make_in_maps(x, wq, wk, wv, wo, *, n_batch_shards, n_head_shards,
                 NQ_TOT, NKV_TOT, HD, TC):
    """Returns list of in_maps, one per core (batch-major: core = b*G + g)."""
    B, S, D = x.shape
    G = n_head_shards
    NQ = NQ_TOT // G
    NKV = NKV_TOT // G
    perm = rope_perm(HD)
    cosT, sinT = rope_tables(S, HD)
    csT = np.concatenate([cosT, sinT], axis=0)  # [HD, S]
    masks = causal_masks(TC)

    # Per-batch xT (shared across head shards)
    xt = {}
    for b in range(B):
        xt[b] = np.ascontiguousarray(x[b].T).astype(BF16)  # [D, S]

    # Per-headgroup weight shards
    wshard = {}
    for g in range(G):
        qrows = slice(g * NQ * HD, (g + 1) * NQ * HD)
        kvrows = slice(g * NKV * HD, (g + 1) * NKV * HD)
        wq_g = wq[qrows, :].copy()      # [NQ*HD, D]
        wk_g = wk[kvrows, :].copy()
        wv_g = wv[kvrows, :].copy()
        # RoPE permutation of output rows, per head
        for hh in range(NQ):
            blk = wq_g[hh * HD:(hh + 1) * HD]
            wq_g[hh * HD:(hh + 1) * HD] = blk[perm]
        for hh in range(NKV):
            blk = wk_g[hh * HD:(hh + 1) * HD]
            wk_g[hh * HD:(hh + 1) * HD] = blk[perm]
        wshard[g] = (
            np.ascontiguousarray(wq_g.T).astype(BF16),   # [D, NQ*HD]
            np.ascontiguousarray(wk_g.T).astype(BF16),
            np.ascontiguousarray(wv_g.T).astype(BF16),
            np.ascontiguousarray(wo[:, qrows].T).astype(BF16),  # [NQ*HD, D]
        )

    in_maps = []
    for b in range(n_batch_shards):
        for g in range(G):
            wqp, wkp, wvp, wot = wshard[g]
            in_maps.append({
                "xb": xt[b],
                "wqp": wqp, "wkp": wkp, "wvp": wvp, "wot": wot,
                "csT": csT,
                "masks": masks,
            })
    return in_maps


def combine_outputs(outTs, B, G):
    """outTs: list of [D, S] partials, core order b*G+g. Returns [B, S, D]."""
    outs = []
    for b in range(B):
        acc = outTs[b * G].astype(np.float32).copy()
        for g in range(1, G):
            acc += outTs[b * G + g]
        outs.append(acc.T)  # [S, D]
    return np.stack(outs)


_NC_CACHE = {}


def _get_nc(S, D, NQ, NKV, HD, TC):
    key = (S, D, NQ, NKV, HD, TC)
    if key not in _NC_CACHE:
        nc = build_attention_nc(S=S, D=D, NQ=NQ, NKV=NKV, HD=HD, TC=TC)
        split_multiwait_insts(nc)
        _NC_CACHE[key] = nc
    return _NC_CACHE[key]


def kernel(**inputs):
    x = np.asarray(inputs["x"], dtype=np.float32)
    wq = np.asarray(inputs["wq"], dtype=np.float32)
    wk = np.asarray(inputs["wk"], dtype=np.float32)
    wv = np.asarray(inputs["wv"], dtype=np.float32)
    wo = np.asarray(inputs["wo"], dtype=np.float32)

    B, S, D = x.shape          # (2, 2048, 2048)
    NQ_TOT = wq.shape[0] // 128
    NKV_TOT = wk.shape[0] // 128
    HD = 128
    TC = 512
    G = 4                      # head shards
    NQ, NKV = NQ_TOT // G, NKV_TOT // G

    nc = _get_nc(S, D, NQ, NKV, HD, TC)
    in_maps = make_in_maps(
        x, wq, wk, wv, wo,
        n_batch_shards=B, n_head_shards=G,
        NQ_TOT=NQ_TOT, NKV_TOT=NKV_TOT, HD=HD, TC=TC,
    )

    from concourse.bass_utils import run_bass_kernel_spmd

    trace = os.environ.get("BASS_ATTN_TRACE") == "1"
    res = run_bass_kernel_spmd(nc, in_maps, list(range(len(in_maps))), trace=trace)
    kernel.last_results = res
    outTs = [r["outT"] for r in res.results]
    return combine_outputs(outTs, B, G).astype(np.float32)


# revision 11
# speedup vs baseline: 2.7193x; 1.0553x over previous
"""Trainium2 Bass kernel for nn_Attention_77043123355775.

Sharded GQA causal attention with RoPE: 8 NeuronCores as 2-way data
parallel (batch) x 4-way tensor parallel (heads). Each core computes its
4 Q heads / 2 KV heads for one batch entry and a partial output
projection (x[b] @ W)^T; the host sums the 4 partials per batch.

All matmuls are single bf16 (inputs rounded to bf16, fp32 PSUM
accumulation), good for ~1e-3 relative error against the 2e-2 gate at
1/3 the tensor-engine cost of hi/lo splitting. Scores are computed
transposed (k on partitions) so the kernel needs no on-chip transposes.
Weights stay resident in SBUF across all token chunks.
"""
import math
import os
import sys

for _p in ("/opt/trn_rl_repo",):
    if _p not in sys.path:
        sys.path.insert(0, _p)

import ml_dtypes
import numpy as np

import concourse.bass as bass
import concourse.mybir as mybir
import concourse.tile as tile

from concourse.tile import add_dep_helper

dt = mybir.dt
AF = mybir.ActivationFunctionType


def build_attention_nc(S=2048, D=2048, NQ=4, NKV=2, HD=128, TC=512):
    assert HD == 128
    C = D // 128          # contraction chunks over features
    TB = S // 128         # 128-token blocks
    NTC = S // TC         # token chunks
    DB = D // 128         # output feature blocks
    CO = NQ * HD // 128   # contraction chunks for wo (= NQ)
    REP = NQ // NKV
    CH = C // 2           # c-chunks per wv half-tile
    CQ = max(C // 4, 1)   # c-chunks per x quarter-tile
    NG = C // CQ
    scale = 1.0 / math.sqrt(HD)

    nc = bass.Bass()

    xb = nc.dram_tensor("xb", [D, S], dt.bfloat16, kind="ExternalInput")
    wqp = nc.dram_tensor("wqp", [D, NQ * HD], dt.bfloat16, kind="ExternalInput")
    wkp = nc.dram_tensor("wkp", [D, NKV * HD], dt.bfloat16, kind="ExternalInput")
    wvp = nc.dram_tensor("wvp", [D, NKV * HD], dt.bfloat16, kind="ExternalInput")
    wot = nc.dram_tensor("wot", [NQ * HD, D], dt.bfloat16, kind="ExternalInput")
    csT = nc.dram_tensor("csT", [HD, S], dt.float32, kind="ExternalInput")
    masks = nc.dram_tensor("masks", [4 * 128, TC], dt.bfloat16, kind="ExternalInput")
    outT = nc.dram_tensor("outT", [D, S], dt.bfloat16, kind="ExternalOutput")

    with tile.TileContext(nc) as tc:
        with (
            tc.tile_pool(name="const", bufs=1) as constp,
            tc.tile_pool(name="tabs", bufs=1) as tabp,
            tc.tile_pool(name="weights", bufs=1) as wp,
            tc.tile_pool(name="acts", bufs=1) as actp,
            tc.tile_pool(name="chunkacts", bufs=1) as cap,
            tc.tile_pool(name="xstream", bufs=2) as xsp,
            tc.tile_pool(name="scratch", bufs=3) as scr,
            tc.tile_pool(name="psum", bufs=1, space="PSUM") as psp,
        ):
            ones_t = constp.tile([128, 1], dt.bfloat16, tag="ones")
            nc.vector.memset(ones_t[:], 1.0)
            ones_row = constp.tile([1, 128], dt.bfloat16, tag="ones_row")
            nc.vector.memset(ones_row[:], 1.0)

            # ---- one-time loads: tables + weights (resident all chunks) ----
            # gpsimd ring: cos/sin, masks, wv, wo; scalar ring: wq, wk.
            cs_t = tabp.tile([HD, S], dt.float32, tag="cs")
            nc.gpsimd.dma_start(cs_t[:], csT[:])
            cos_t = cs_t[0:HD // 2, :]
            sin_t = cs_t[HD // 2:HD, :]
            mask_t = [tabp.tile([128, TC], dt.bfloat16, tag=f"mask{i}", name=f"mask{i}") for i in range(4)]
            for i in range(4):
                nc.gpsimd.dma_start(mask_t[i][:], masks[i * 128:(i + 1) * 128, :])

            wq_t = []
            for h in range(NQ):
                t = wp.tile([128, C * HD], dt.bfloat16, tag=f"wq{h}", name=f"wq{h}")
                nc.scalar.dma_start(
                    t.rearrange("p (c n) -> p c n", c=C),
                    wqp[:, h * HD:(h + 1) * HD].rearrange("(c p) n -> p c n", p=128),
                )
                wq_t.append(t)
            wk_t = []
            for h in range(NKV):
                t = wp.tile([128, C * HD], dt.bfloat16, tag=f"wk{h}", name=f"wk{h}")
                nc.scalar.dma_start(
                    t.rearrange("p (c n) -> p c n", c=C),
                    wkp[:, h * HD:(h + 1) * HD].rearrange("(c p) n -> p c n", p=128),
                )
                wk_t.append(t)
            # wv/wo loads are gated on early chunk-0 matmuls (below) so the
            # startup HBM burst stays focused on x/wq/wk/csT
            wv_g = []
            wv_dma = []
            for g in range(2):
                rs = slice(g * CH * 128, (g + 1) * CH * 128)
                t = wp.tile([128, CH * NKV * HD], dt.bfloat16, tag=f"wv{g}", name=f"wv{g}")
                wv_dma.append(nc.gpsimd.dma_start(
                    t.rearrange("p (c n) -> p c n", c=CH),
                    wvp[rs, :].rearrange("(c p) n -> p c n", p=128),
                ))
                wv_g.append(t)
            wo_t = wp.tile([128, CO * D], dt.bfloat16, tag="wo")
            wo_dma = []
            for g in range(2):
                wo_dma.append(nc.gpsimd.dma_start(
                    wo_t.rearrange("p (c n) -> p c n", c=CO)[:, g * (CO // 2):(g + 1) * (CO // 2), :],
                    wot[g * (CO // 2) * 128:(g + 1) * (CO // 2) * 128, :].rearrange("(c p) n -> p c n", p=128),
                ))

            # K persists for the full sequence (written chunk by chunk);
            # V persists per 128-token block
            kth = [actp.tile([128, S], dt.bfloat16, tag=f"kth{h}", name=f"kth{h}") for h in range(NKV)]
            vh_t = [actp.tile([128, NKV * HD], dt.bfloat16, tag=f"vh{b}", name=f"vh{b}") for b in range(TB)]

            # x chunk quarters, double buffered across chunks (prefetch)
            x_tiles = {}

            def emit_x_load(tci):
                ts_ = slice(tci * TC, (tci + 1) * TC)
                g_tiles = []
                for g in range(NG):
                    rs = slice(g * CQ * 128, (g + 1) * CQ * 128)
                    t = xsp.tile([128, CQ * TC], dt.bfloat16, tag="xh",
                                 bufs=2 * NG, name=f"xh_{tci}_{g}")
                    nc.sync.dma_start(
                        t.rearrange("p (c n) -> p c n", c=CQ),
                        xb[rs, ts_].rearrange("(c p) n -> p c n", p=128),
                    )
                    g_tiles.append(t)
                x_tiles[tci] = g_tiles

            emit_x_load(0)

            for tci in range(NTC):
                ts = slice(tci * TC, (tci + 1) * TC)
                xh_g = x_tiles.pop(tci)
                qth = [cap.tile([128, TC], dt.bfloat16, tag=f"qth{h}", name=f"qth{h}_{tci}") for h in range(NQ)]
                oth = [cap.tile([128, TC], dt.bfloat16, tag=f"oth{h}", name=f"oth{h}_{tci}") for h in range(NQ)]

                def xh_c(c):
                    return xh_g[c // CQ][:, (c % CQ) * TC:(c % CQ + 1) * TC]

                # ---- QKV projections + RoPE ----
                for h in range(NQ + NKV):
                    is_q = h < NQ
                    wt = wq_t[h] if is_q else wk_t[h - NQ]
                    ps = psp.tile([128, TC], dt.float32, tag="mm", bufs=2)
                    for c in range(C):
                        mm = nc.tensor.matmul(
                            ps[:], wt[:, c * HD:(c + 1) * HD], xh_c(c),
                            start=(c == 0), stop=(c == C - 1),
                        )
                        if tci == 0 and c == 0 and h in (2, 3):
                            add_dep_helper(wv_dma[h - 2].ins, mm.ins,
                                           reason="startup prefetch throttle")
                    # RoPE in f32 from PSUM; DVE does the 4 products,
                    # gpsimd combines into the bf16 destination
                    cs = cos_t[:, ts]
                    sn = sin_t[:, ts]
                    xr = ps[0:64, :]
                    xi = ps[64:128, :]
                    # products placed so each combine's operands share base
                    # partitions (SB+SB ops require equal base partition)
                    ta = scr.tile([128, TC], dt.float32, tag="ropetmp", bufs=2)
                    tb = scr.tile([128, TC], dt.float32, tag="ropetmp2", bufs=2)
                    nc.vector.tensor_tensor(ta[0:64, :], xr, cs, mybir.AluOpType.mult)
                    nc.vector.tensor_tensor(tb[0:64, :], xi, sn, mybir.AluOpType.mult)
                    nc.vector.tensor_tensor(ta[64:128, :], xr, sn, mybir.AluOpType.mult)
                    nc.vector.tensor_tensor(tb[64:128, :], xi, cs, mybir.AluOpType.mult)
                    dsth = qth[h][:] if is_q else kth[h - NQ][:, ts]
                    nc.gpsimd.tensor_tensor(dsth[0:64, :], ta[0:64, :], tb[0:64, :], mybir.AluOpType.subtract)
                    nc.gpsimd.tensor_tensor(dsth[64:128, :], ta[64:128, :], tb[64:128, :], mybir.AluOpType.add)

                # ---- V projection ----
                for tb_i in range(TC // 128):
                    tbg = tci * (TC // 128) + tb_i
                    ps = psp.tile([128, NKV * HD], dt.float32, tag="mm", bufs=2)
                    for c in range(C):
                        xh_s = xh_c(c)[:, tb_i * 128:(tb_i + 1) * 128]
                        g, cc = c // CH, c % CH
                        mm = nc.tensor.matmul(
                            ps[:], xh_s, wv_g[g][:, cc * NKV * HD:(cc + 1) * NKV * HD],
                            start=(c == 0), stop=(c == C - 1),
                        )
                        if tci == 0 and tb_i == 0 and c == 0:
                            for wd in wo_dma:
                                add_dep_helper(wd.ins, mm.ins,
                                               reason="startup prefetch throttle")
                    nc.vector.tensor_copy(vh_t[tbg][:], ps[:])

                # prefetch next chunk's x while attention runs
                if tci + 1 < NTC:
                    emit_x_load(tci + 1)

                # ---- attention for q-chunk tci (keys 0..(tci+1)*TC) ----
                qc = tci
                nkb = (qc + 1) * (TC // 128)

                # Two-stage software pipeline over all (head, block)
                # pairs: scores/exp/sum lead PV by LAG blocks so the PE
                # never waits on the ACT/DVE probs chain at head starts,
                # and each head's 1/sum chain resolves while its last few
                # PV blocks are still streaming.
                LAG = 4
                blocks = [(h, kb) for h in range(NQ) for kb in range(nkb)]
                head_ot = {}
                head_sum = {}
                head_recb = {}

                def emit_scores(h, kb):
                    kv = h // REP
                    d = kb * 128 - qc * TC
                    ks = slice(kb * 128, (kb + 1) * 128)
                    q0 = max(d, 0)
                    sc_ps = psp.tile([128, TC], dt.float32, tag="mm", bufs=2,
                                     name=f"sc_{tci}_{h}_{kb}")
                    nc.tensor.matmul(sc_ps[:, q0:TC], kth[kv][:, ks], qth[h][:, q0:TC],
                                     start=True, stop=True)
                    ph = scr.tile([128, TC], dt.bfloat16, tag="ph", bufs=LAG + 2,
                                  name=f"ph_{tci}_{h}_{kb}")
                    nc.scalar.activation(ph[:, q0:TC], sc_ps[:, q0:TC], AF.Exp, bias=0.0, scale=scale)
                    if d >= 0:
                        nc.vector.tensor_tensor(ph[:, q0:TC], ph[:, q0:TC], mask_t[d // 128][:, q0:TC], mybir.AluOpType.mult)
                    if kb == 0:
                        head_sum[h] = psp.tile([1, TC], dt.float32, tag="sums", bufs=2,
                                               name=f"sum_{tci}_{h}")
                    sum_ps = head_sum[h]
                    nc.tensor.matmul(
                        sum_ps[:, q0:TC], ones_t[:], ph[:, q0:TC],
                        start=(kb == 0), stop=(kb == nkb - 1),
                    )
                    if kb == nkb - 1:
                        # 1/sum as exp(-ln(sum)) on the ACT engine: ~1e-3 rel,
                        # far cheaper than the exact DVE reciprocal (3.3us)
                        lns = scr.tile([1, TC], dt.float32, tag="lns", bufs=2, name=f"lns_{tci}_{h}")
                        nc.scalar.activation(lns[:], sum_ps[:], AF.Ln, bias=0.0, scale=1.0)
                        rec16 = scr.tile([1, TC], dt.bfloat16, tag="rec16", bufs=2, name=f"rec16_{tci}_{h}")
                        nc.scalar.activation(rec16[:], lns[:], AF.Exp, bias=0.0, scale=-1.0)
                        bc_ps = psp.tile([128, TC], dt.float32, tag="bcast", bufs=1, name=f"bc_{tci}_{h}")
                        nc.tensor.matmul(bc_ps[:], ones_row[:], rec16[:], start=True, stop=True)
                        recb = scr.tile([128, TC], dt.float32, tag="recb", bufs=2, name=f"recb_{tci}_{h}")
                        nc.scalar.copy(recb[:], bc_ps[:])
                        head_recb[h] = recb
                    return ph

                def emit_pv(h, kb, ph):
                    kv = h // REP
                    vcol = kv * HD
                    d = kb * 128 - qc * TC
                    q0 = max(d, 0)
                    if kb == 0:
                        head_ot[h] = psp.tile([128, TC], dt.float32, tag="otps", bufs=3,
                                              name=f"ot_{tci}_{h}")
                    ot_ps = head_ot[h]
                    nc.tensor.matmul(
                        ot_ps[:, q0:TC], vh_t[kb][:, vcol:vcol + HD], ph[:, q0:TC],
                        start=(kb == 0), stop=(kb == nkb - 1),
                    )
                    if kb == nkb - 1:
                        nc.vector.tensor_tensor(oth[h][:], ot_ps[:], head_recb[h][:], mybir.AluOpType.mult)

                probs_q = []
                for h, kb in blocks:
                    probs_q.append((h, kb, emit_scores(h, kb)))
                    if len(probs_q) > LAG:
                        hh, kk, ph = probs_q.pop(0)
                        emit_pv(hh, kk, ph)
                for hh, kk, ph in probs_q:
                    emit_pv(hh, kk, ph)

                # ---- output projection for token-chunk tci ----
                # o3 copies on ACT, stores round-robin on gpsimd/scalar rings
                # (sync stays clear for the next chunk's x prefetch)
                for db in range(DB):
                    ds_ = slice(db * 128, (db + 1) * 128)
                    ps = psp.tile([128, TC], dt.float32, tag="mm", bufs=2)
                    for c in range(CO):
                        nc.tensor.matmul(
                            ps[:], wo_t[:, c * D + db * 128:c * D + (db + 1) * 128], oth[c][:],
                            start=(c == 0), stop=(c == CO - 1),
                        )
                    o3 = scr.tile([128, TC], dt.bfloat16, tag="o3", bufs=3)
                    nc.scalar.copy(o3[:], ps[:])
                    eng = nc.gpsimd if db % 2 == 0 else nc.scalar
                    eng.dma_start(outT[ds_, ts], o3[:])

    return nc


# ---------------------------------------------------------------------------
# walrus in this container refuses >1 sem wait per instruction ("Too many
# sync wait commands"). Hoist excess waits onto same-engine NoOps inserted
# immediately before the instruction - program order on the engine queue
# preserves the sync semantics.
def split_multiwait_insts(nc, max_waits=1):
    n_split = 0
    for bb in nc.main_func.blocks:
        insts = bb.instructions
        i = 0
        while i < len(insts):
            ins = insts[i]
            si = getattr(ins, "sync_info", None)
            if si is not None and si.on_wait and len(si.on_wait) > max_waits:
                waits = list(si.on_wait)
                head, tail = waits[:-max_waits], waits[-max_waits:]
                nops = []
                for j in range(0, len(head), max_waits):
                    nop = mybir.InstNoOp(name=f"{ins.name}-ws{j}", ins=[], outs=[])
                    nop.engine = ins.engine
                    nop.sync_info = mybir.SyncInfo(
                        on_wait=head[j:j + max_waits], on_update=[])
                    nops.append(nop)
                ins.sync_info = mybir.SyncInfo(
                    on_wait=tail, on_update=list(si.on_update or []))
                insts[i:i] = nops
                i += len(nops)
                n_split += 1
            i += 1
    return n_split


# ---------------------------------------------------------------------------
# Host-side shard preparation / gather
BF16 = ml_dtypes.bfloat16


def rope_tables(S, HD):
    inv = 1.0 / (10000.0 ** (np.arange(0, HD, 2, dtype=np.float32) / HD))
    t = np.arange(S, dtype=np.float32)
    f = np.outer(t, inv).astype(np.float32)  # [S, HD//2]
    return np.ascontiguousarray(np.cos(f).T), np.ascontiguousarray(np.sin(f).T)


def causal_masks(TC):
    # masks[dd][k, qrel] = 1 if k + dd*128 <= qrel else 0
    out = np.zeros((4 * 128, TC), BF16)
    k = np.arange(128)[:, None]
    q = np.arange(TC)[None, :]
    for dd in range(4):
        out[dd * 128:(dd + 1) * 128] = (k + dd * 128 <= q).astype(BF16)
    return out


def rope_perm(HD):
    # new row i (i < HD//2) = old 2i; new row HD//2+i = old 2i+1
    return np.concatenate([np.arange(0, HD, 2), np.arange(1, HD, 2)])


def make_in_maps(x, wq, wk, wv, wo, *, n_batch_shards, n_head_shards,
                 NQ_TOT, NKV_TOT, HD, TC):
    """Returns list of in_maps, one per core (batch-major: core = b*G + g)."""
    B, S, D = x.shape
    G = n_head_shards
    NQ = NQ_TOT // G
    NKV = NKV_TOT // G
    perm = rope_perm(HD)
    cosT, sinT = rope_tables(S, HD)
    csT = np.concatenate([cosT, sinT], axis=0)  # [HD, S]
    masks = causal_masks(TC)

    # Per-batch xT (shared across head shards)
    xt = {}
    for b in range(B):
        xt[b] = np.ascontiguousarray(x[b].T).astype(BF16)  # [D, S]

    # Per-headgroup weight shards
    wshard = {}
    for g in range(G):
        qrows = slice(g * NQ * HD, (g + 1) * NQ * HD)
        kvrows = slice(g * NKV * HD, (g + 1) * NKV * HD)
        wq_g = wq[qrows, :].copy()      # [NQ*HD, D]
        wk_g = wk[kvrows, :].copy()
        wv_g = wv[kvrows, :].copy()
        # RoPE permutation of output rows, per head
        for hh in range(NQ):
            blk = wq_g[hh * HD:(hh + 1) * HD]
            wq_g[hh * HD:(hh + 1) * HD] = blk[perm]
        for hh in range(NKV):
            blk = wk_g[hh * HD:(hh + 1) * HD]
            wk_g[hh * HD:(hh + 1) * HD] = blk[perm]
        wshard[g] = (
            np.ascontiguousarray(wq_g.T).astype(BF16),   # [D, NQ*HD]
            np.ascontiguousarray(wk_g.T).astype(BF16),
            np.ascontiguousarray(wv_g.T).astype(BF16),
            np.ascontiguousarray(wo[:, qrows].T).astype(BF16),  # [NQ*HD, D]
        )

    in_maps = []
    for b in range(n_batch_shards):
        for g in range(G):
            wqp, wkp, wvp, wot = wshard[g]
            in_maps.append({
                "xb": xt[b],
                "wqp": wqp, "wkp": wkp, "wvp": wvp, "wot": wot,
                "csT": csT,
                "masks": masks,
            })
    return in_maps


def combine_outputs(outTs, B, G):
    """outTs: list of [D, S] partials, core order b*G+g. Returns [B, S, D]."""
    outs = []
    for b in range(B):
        acc = outTs[b * G].astype(np.float32).copy()
        for g in range(1, G):
            acc += outTs[b * G + g]
        outs.append(acc.T)  # [S, D]
    return np.stack(outs)


_NC_CACHE = {}


def _get_nc(S, D, NQ, NKV, HD, TC):
    key = (S, D, NQ, NKV, HD, TC)
    if key not in _NC_CACHE:
        nc = build_attention_nc(S=S, D=D, NQ=NQ, NKV=NKV, HD=HD, TC=TC)
        split_multiwait_insts(nc)
        _NC_CACHE[key] = nc
    return _NC_CACHE[key]


def kernel(**inputs):
    x = np.asarray(inputs["x"], dtype=np.float32)
    wq = np.asarray(inputs["wq"], dtype=np.float32)
    wk = np.asarray(inputs["wk"], dtype=np.float32)
    wv = np.asarray(inputs["wv"], dtype=np.float32)
    wo = np.asarray(inputs["wo"], dtype=np.float32)

    B, S, D = x.shape          # (2, 2048, 2048)
    NQ_TOT = wq.shape[0] // 128
    NKV_TOT = wk.shape[0] // 128
    HD = 128
    TC = 512
    G = 4                      # head shards
    NQ, NKV = NQ_TOT // G, NKV_TOT // G

    nc = _get_nc(S, D, NQ, NKV, HD, TC)
    in_maps = make_in_maps(
        x, wq, wk, wv, wo,
        n_batch_shards=B, n_head_shards=G,
        NQ_TOT=NQ_TOT, NKV_TOT=NKV_TOT, HD=HD, TC=TC,
    )

    from concourse.bass_utils import run_bass_kernel_spmd

    trace = os.environ.get("BASS_ATTN_TRACE") == "1"
    res = run_bass_kernel_spmd(nc, in_maps, list(range(len(in_maps))), trace=trace)
    kernel.last_results = res
    outTs = [r["outT"] for r in res.results]
    return combine_outputs(outTs, B, G).astype(np.float32)


# revision 16
# speedup vs baseline: 2.7217x; 1.0009x over previous
"""Trainium2 Bass kernel for nn_Attention_77043123355775.

Sharded GQA causal attention with RoPE: 8 NeuronCores as 2-way data
parallel (batch) x 4-way tensor parallel (heads). Each core computes its
4 Q heads / 2 KV heads for one batch entry and a partial output
projection (x[b] @ W)^T; the host sums the 4 partials per batch.

All matmuls are single bf16 (inputs rounded to bf16, fp32 PSUM
accumulation), good for ~1e-3 relative error against the 2e-2 gate at
1/3 the tensor-engine cost of hi/lo splitting. Scores are computed
transposed (k on partitions) so the kernel needs no on-chip transposes.
Weights stay resident in SBUF across all token chunks.
"""
import math
import os
import sys

for _p in ("/opt/trn_rl_repo",):
    if _p not in sys.path:
        sys.path.insert(0, _p)

import ml_dtypes
import numpy as np

import concourse.bass as bass
import concourse.mybir as mybir
import concourse.tile as tile

from concourse.tile import add_dep_helper

dt = mybir.dt
AF = mybir.ActivationFunctionType


def build_attention_nc(S=2048, D=2048, NQ=4, NKV=2, HD=128, TC=512):
    assert HD == 128
    C = D // 128          # contraction chunks over features
    TB = S // 128         # 128-token blocks
    NTC = S // TC         # token chunks
    DB = D // 128         # output feature blocks
    CO = NQ * HD // 128   # contraction chunks for wo (= NQ)
    REP = NQ // NKV
    CH = C // 2           # c-chunks per wv half-tile
    CQ = max(C // 4, 1)   # c-chunks per x quarter-tile
    NG = C // CQ
    scale = 1.0 / math.sqrt(HD)

    nc = bass.Bass()

    xb = nc.dram_tensor("xb", [D, S], dt.bfloat16, kind="ExternalInput")
    wqp = nc.dram_tensor("wqp", [D, NQ * HD], dt.bfloat16, kind="ExternalInput")
    wkp = nc.dram_tensor("wkp", [D, NKV * HD], dt.bfloat16, kind="ExternalInput")
    wvp = nc.dram_tensor("wvp", [D, NKV * HD], dt.bfloat16, kind="ExternalInput")
    wot = nc.dram_tensor("wot", [NQ * HD, D], dt.bfloat16, kind="ExternalInput")
    csT = nc.dram_tensor("csT", [HD, S], dt.float32, kind="ExternalInput")
    masks = nc.dram_tensor("masks", [4 * 128, TC], dt.bfloat16, kind="ExternalInput")
    outT = nc.dram_tensor("outT", [D, S], dt.bfloat16, kind="ExternalOutput")

    with tile.TileContext(nc) as tc:
        with (
            tc.tile_pool(name="const", bufs=1) as constp,
            tc.tile_pool(name="tabs", bufs=1) as tabp,
            tc.tile_pool(name="weights", bufs=1) as wp,
            tc.tile_pool(name="acts", bufs=1) as actp,
            tc.tile_pool(name="chunkacts", bufs=1) as cap,
            tc.tile_pool(name="xstream", bufs=2) as xsp,
            tc.tile_pool(name="scratch", bufs=3) as scr,
            tc.tile_pool(name="psum", bufs=1, space="PSUM") as psp,
        ):
            ones_t = constp.tile([128, 1], dt.bfloat16, tag="ones")
            nc.vector.memset(ones_t[:], 1.0)
            ones_row = constp.tile([1, 128], dt.bfloat16, tag="ones_row")
            nc.vector.memset(ones_row[:], 1.0)

            # ---- one-time loads: tables + weights (resident all chunks) ----
            # gpsimd ring: cos/sin, masks, wv, wo; scalar ring: wq, wk.
            # cos/sin: chunk-0 columns land first; the rest is gated off the
            # startup burst (needed only from chunk 1 onwards)
            cs_t = tabp.tile([HD, S], dt.float32, tag="cs")
            nc.gpsimd.dma_start(cs_t[:, 0:TC], csT[:, 0:TC])
            cos_t = cs_t[0:HD // 2, :]
            sin_t = cs_t[HD // 2:HD, :]
            mask_t = [tabp.tile([128, TC], dt.bfloat16, tag=f"mask{i}", name=f"mask{i}") for i in range(4)]

            wq_t = []
            for h in range(NQ):
                t = wp.tile([128, C * HD], dt.bfloat16, tag=f"wq{h}", name=f"wq{h}")
                nc.scalar.dma_start(
                    t.rearrange("p (c n) -> p c n", c=C),
                    wqp[:, h * HD:(h + 1) * HD].rearrange("(c p) n -> p c n", p=128),
                )
                wq_t.append(t)
            wk_t = []
            for h in range(NKV):
                t = wp.tile([128, C * HD], dt.bfloat16, tag=f"wk{h}", name=f"wk{h}")
                nc.scalar.dma_start(
                    t.rearrange("p (c n) -> p c n", c=C),
                    wkp[:, h * HD:(h + 1) * HD].rearrange("(c p) n -> p c n", p=128),
                )
                wk_t.append(t)
            # wv/masks/csT-rest/wo loads are gated on early chunk-0 matmuls
            # (below) so the startup HBM burst stays focused on x/wq/wk/csT0.
            # gpsimd ring order matches gating order (rings execute in order).
            wv_g = []
            wv_dma = []
            for g in range(2):
                rs = slice(g * CH * 128, (g + 1) * CH * 128)
                t = wp.tile([128, CH * NKV * HD], dt.bfloat16, tag=f"wv{g}", name=f"wv{g}")
                wv_dma.append(nc.gpsimd.dma_start(
                    t.rearrange("p (c n) -> p c n", c=CH),
                    wvp[rs, :].rearrange("(c p) n -> p c n", p=128),
                ))
                wv_g.append(t)
            mask_dma = []
            for i in range(4):
                mask_dma.append(nc.gpsimd.dma_start(mask_t[i][:], masks[i * 128:(i + 1) * 128, :]))
            cs_rest_dma = nc.gpsimd.dma_start(cs_t[:, TC:S], csT[:, TC:S])
            wo_t = wp.tile([128, CO * D], dt.bfloat16, tag="wo")
            wo_dma = []
            for g in range(2):
                wo_dma.append(nc.gpsimd.dma_start(
                    wo_t.rearrange("p (c n) -> p c n", c=CO)[:, g * (CO // 2):(g + 1) * (CO // 2), :],
                    wot[g * (CO // 2) * 128:(g + 1) * (CO // 2) * 128, :].rearrange("(c p) n -> p c n", p=128),
                ))

            # K persists for the full sequence (written chunk by chunk);
            # V persists per 128-token block
            kth = [actp.tile([128, S], dt.bfloat16, tag=f"kth{h}", name=f"kth{h}") for h in range(NKV)]
            vh_t = [actp.tile([128, NKV * HD], dt.bfloat16, tag=f"vh{b}", name=f"vh{b}") for b in range(TB)]

            # x chunk quarters, double buffered across chunks (prefetch)
            x_tiles = {}

            def emit_x_load(tci):
                ts_ = slice(tci * TC, (tci + 1) * TC)
                g_tiles = []
                for g in range(NG):
                    rs = slice(g * CQ * 128, (g + 1) * CQ * 128)
                    t = xsp.tile([128, CQ * TC], dt.bfloat16, tag="xh",
                                 bufs=2 * NG, name=f"xh_{tci}_{g}")
                    nc.sync.dma_start(
                        t.rearrange("p (c n) -> p c n", c=CQ),
                        xb[rs, ts_].rearrange("(c p) n -> p c n", p=128),
                    )
                    g_tiles.append(t)
                x_tiles[tci] = g_tiles

            emit_x_load(0)

            for tci in range(NTC):
                ts = slice(tci * TC, (tci + 1) * TC)
                xh_g = x_tiles.pop(tci)
                qth = [cap.tile([128, TC], dt.bfloat16, tag=f"qth{h}", name=f"qth{h}_{tci}") for h in range(NQ)]
                oth = [cap.tile([128, TC], dt.bfloat16, tag=f"oth{h}", name=f"oth{h}_{tci}") for h in range(NQ)]

                def xh_c(c):
                    return xh_g[c // CQ][:, (c % CQ) * TC:(c % CQ + 1) * TC]

                # ---- QKV projections + RoPE ----
                for h in range(NQ + NKV):
                    is_q = h < NQ
                    wt = wq_t[h] if is_q else wk_t[h - NQ]
                    ps = psp.tile([128, TC], dt.float32, tag="mm", bufs=2)
                    for c in range(C):
                        mm = nc.tensor.matmul(
                            ps[:], wt[:, c * HD:(c + 1) * HD], xh_c(c),
                            start=(c == 0), stop=(c == C - 1),
                        )
                        if tci == 0 and c == 0:
                            if h == 1:
                                add_dep_helper(wv_dma[0].ins, mm.ins,
                                               reason="startup prefetch throttle")
                            elif h == 2:
                                add_dep_helper(wv_dma[1].ins, mm.ins,
                                               reason="startup prefetch throttle")
                                for md in mask_dma:
                                    add_dep_helper(md.ins, mm.ins,
                                                   reason="startup prefetch throttle")
                            elif h == 3:
                                add_dep_helper(cs_rest_dma.ins, mm.ins,
                                               reason="startup prefetch throttle")
                    # RoPE in f32 from PSUM; DVE does the 4 products,
                    # gpsimd combines into the bf16 destination
                    cs = cos_t[:, ts]
                    sn = sin_t[:, ts]
                    xr = ps[0:64, :]
                    xi = ps[64:128, :]
                    # products placed so each combine's operands share base
                    # partitions (SB+SB ops require equal base partition)
                    ta = scr.tile([128, TC], dt.float32, tag="ropetmp", bufs=2)
                    tb = scr.tile([128, TC], dt.float32, tag="ropetmp2", bufs=2)
                    nc.vector.tensor_tensor(ta[0:64, :], xr, cs, mybir.AluOpType.mult)
                    nc.vector.tensor_tensor(tb[0:64, :], xi, sn, mybir.AluOpType.mult)
                    nc.vector.tensor_tensor(ta[64:128, :], xr, sn, mybir.AluOpType.mult)
                    nc.vector.tensor_tensor(tb[64:128, :], xi, cs, mybir.AluOpType.mult)
                    dsth = qth[h][:] if is_q else kth[h - NQ][:, ts]
                    nc.gpsimd.tensor_tensor(dsth[0:64, :], ta[0:64, :], tb[0:64, :], mybir.AluOpType.subtract)
                    nc.gpsimd.tensor_tensor(dsth[64:128, :], ta[64:128, :], tb[64:128, :], mybir.AluOpType.add)

                # ---- V projection ----
                for tb_i in range(TC // 128):
                    tbg = tci * (TC // 128) + tb_i
                    ps = psp.tile([128, NKV * HD], dt.float32, tag="mm", bufs=2)
                    for c in range(C):
                        xh_s = xh_c(c)[:, tb_i * 128:(tb_i + 1) * 128]
                        g, cc = c // CH, c % CH
                        mm = nc.tensor.matmul(
                            ps[:], xh_s, wv_g[g][:, cc * NKV * HD:(cc + 1) * NKV * HD],
                            start=(c == 0), stop=(c == C - 1),
                        )
                        if tci == 0 and tb_i == 0 and c == 0:
                            for wd in wo_dma:
                                add_dep_helper(wd.ins, mm.ins,
                                               reason="startup prefetch throttle")
                    nc.vector.tensor_copy(vh_t[tbg][:], ps[:])

                # prefetch next chunk's x while attention runs
                if tci + 1 < NTC:
                    emit_x_load(tci + 1)

                # ---- attention for q-chunk tci (keys 0..(tci+1)*TC) ----
                qc = tci
                nkb = (qc + 1) * (TC // 128)

                # Two-stage software pipeline over all (head, block)
                # pairs: scores/exp/sum lead PV by LAG blocks so the PE
                # never waits on the ACT/DVE probs chain at head starts,
                # and each head's 1/sum chain resolves while its last few
                # PV blocks are still streaming.
                LAG = 8
                blocks = [(h, kb) for h in range(NQ) for kb in range(nkb)]
                head_ot = {}
                head_sum = {}
                head_rec16 = {}
                head_recb = {}

                def emit_scores(h, kb):
                    kv = h // REP
                    d = kb * 128 - qc * TC
                    ks = slice(kb * 128, (kb + 1) * 128)
                    q0 = max(d, 0)
                    sc_ps = psp.tile([128, TC], dt.float32, tag="mm", bufs=2,
                                     name=f"sc_{tci}_{h}_{kb}")
                    nc.tensor.matmul(sc_ps[:, q0:TC], kth[kv][:, ks], qth[h][:, q0:TC],
                                     start=True, stop=True)
                    ph = scr.tile([128, TC], dt.bfloat16, tag="ph", bufs=LAG + 2,
                                  name=f"ph_{tci}_{h}_{kb}")
                    nc.scalar.activation(ph[:, q0:TC], sc_ps[:, q0:TC], AF.Exp, bias=0.0, scale=scale)
                    if d >= 0:
                        nc.vector.tensor_tensor(ph[:, q0:TC], ph[:, q0:TC], mask_t[d // 128][:, q0:TC], mybir.AluOpType.mult)
                    if kb == 0:
                        head_sum[h] = psp.tile([1, TC], dt.float32, tag="sums", bufs=2,
                                               name=f"sum_{tci}_{h}")
                    sum_ps = head_sum[h]
                    nc.tensor.matmul(
                        sum_ps[:, q0:TC], ones_t[:], ph[:, q0:TC],
                        start=(kb == 0), stop=(kb == nkb - 1),
                    )
                    if kb == nkb - 1:
                        # 1/sum as exp(-ln(sum)) on the ACT engine: ~1e-3 rel,
                        # far cheaper than the exact DVE reciprocal (3.3us)
                        lns = scr.tile([1, TC], dt.float32, tag="lns", bufs=2, name=f"lns_{tci}_{h}")
                        nc.scalar.activation(lns[:], sum_ps[:], AF.Ln, bias=0.0, scale=1.0)
                        rec16 = scr.tile([1, TC], dt.bfloat16, tag="rec16", bufs=2, name=f"rec16_{tci}_{h}")
                        nc.scalar.activation(rec16[:], lns[:], AF.Exp, bias=0.0, scale=-1.0)
                        head_rec16[h] = rec16
                    return ph

                def emit_pv(h, kb, ph):
                    kv = h // REP
                    vcol = kv * HD
                    d = kb * 128 - qc * TC
                    q0 = max(d, 0)
                    if kb == 0:
                        head_ot[h] = psp.tile([128, TC], dt.float32, tag="otps", bufs=3,
                                              name=f"ot_{tci}_{h}")
                    ot_ps = head_ot[h]
                    nc.tensor.matmul(
                        ot_ps[:, q0:TC], vh_t[kb][:, vcol:vcol + HD], ph[:, q0:TC],
                        start=(kb == 0), stop=(kb == nkb - 1),
                    )
                    if kb == nkb - 2:
                        # broadcast 1/sum now: the ACT ln/exp chain (issued at
                        # scores-lead, LAG blocks ago) has drained, so this
                        # matmul doesn't block the in-order PE queue
                        bc_ps = psp.tile([128, TC], dt.float32, tag="bcast", bufs=1, name=f"bc_{tci}_{h}")
                        nc.tensor.matmul(bc_ps[:], ones_row[:], head_rec16[h][:], start=True, stop=True)
                        recb = scr.tile([128, TC], dt.float32, tag="recb", bufs=2, name=f"recb_{tci}_{h}")
                        nc.scalar.copy(recb[:], bc_ps[:])
                        head_recb[h] = recb
                    if kb == nkb - 1:
                        nc.vector.tensor_tensor(oth[h][:], ot_ps[:], head_recb[h][:], mybir.AluOpType.mult)

                probs_q = []
                for h, kb in blocks:
                    probs_q.append((h, kb, emit_scores(h, kb)))
                    if len(probs_q) > LAG:
                        hh, kk, ph = probs_q.pop(0)
                        emit_pv(hh, kk, ph)
                for hh, kk, ph in probs_q:
                    emit_pv(hh, kk, ph)

                # ---- output projection for token-chunk tci ----
                # o3 copies on ACT, stores round-robin on gpsimd/scalar rings
                # (sync stays clear for the next chunk's x prefetch)
                for db in range(DB):
                    ds_ = slice(db * 128, (db + 1) * 128)
                    ps = psp.tile([128, TC], dt.float32, tag="mm", bufs=2)
                    for c in range(CO):
                        nc.tensor.matmul(
                            ps[:], wo_t[:, c * D + db * 128:c * D + (db + 1) * 128], oth[c][:],
                            start=(c == 0), stop=(c == CO - 1),
                        )
                    o3 = scr.tile([128, TC], dt.bfloat16, tag="o3", bufs=3)
                    nc.scalar.copy(o3[:], ps[:])
                    eng = nc.gpsimd if db % 2 == 0 else nc.scalar
                    eng.dma_start(outT[ds_, ts], o3[:])

    return nc


# ---------------------------------------------------------------------------
# walrus in this container refuses >1 sem wait per instruction ("Too many
# sync wait commands"). Hoist excess waits onto same-engine NoOps inserted
# immediately before the instruction - program order on the engine queue
# preserves the sync semantics.
def split_multiwait_insts(nc, max_waits=1):
    n_split = 0
    for bb in nc.main_func.blocks:
        insts = bb.instructions
        i = 0
        while i < len(insts):
            ins = insts[i]
            si = getattr(ins, "sync_info", None)
            if si is not None and si.on_wait and len(si.on_wait) > max_waits:
                waits = list(si.on_wait)
                head, tail = waits[:-max_waits], waits[-max_waits:]
                nops = []
                for j in range(0, len(head), max_waits):
                    nop = mybir.InstNoOp(name=f"{ins.name}-ws{j}", ins=[], outs=[])
                    nop.engine = ins.engine
                    nop.sync_info = mybir.SyncInfo(
                        on_wait=head[j:j + max_waits], on_update=[])
                    nops.append(nop)
                ins.sync_info = mybir.SyncInfo(
                    on_wait=tail, on_update=list(si.on_update or []))
                insts[i:i] = nops
                i += len(nops)
                n_split += 1
            i += 1
    return n_split


# ---------------------------------------------------------------------------
# Host-side shard preparation / gather
BF16 = ml_dtypes.bfloat16


def rope_tables(S, HD):
    inv = 1.0 / (10000.0 ** (np.arange(0, HD, 2, dtype=np.float32) / HD))
    t = np.arange(S, dtype=np.float32)
    f = np.outer(t, inv).astype(np.float32)  # [S, HD//2]
    return np.ascontiguousarray(np.cos(f).T), np.ascontiguousarray(np.sin(f).T)


def causal_masks(TC):
    # masks[dd][k, qrel] = 1 if k + dd*128 <= qrel else 0
    out = np.zeros((4 * 128, TC), BF16)
    k = np.arange(128)[:, None]
    q = np.arange(TC)[None, :]
    for dd in range(4):
        out[dd * 128:(dd + 1) * 128] = (k + dd * 128 <= q).astype(BF16)
    return out


def rope_perm(HD):
    # new row i (i < HD//2) = old 2i; new row HD//2+i = old 2i+1
    return np.concatenate([np.arange(0, HD, 2), np.arange(1, HD, 2)])


def make_in_maps(x, wq, wk, wv, wo, *, n_batch_shards, n_head_shards,
                 NQ_TOT, NKV_TOT, HD, TC):
    """Returns list of in_maps, one per core (batch-major: core = b*G + g)."""
    B, S, D = x.shape
    G = n_head_shards
    NQ = NQ_TOT // G
    NKV = NKV_TOT // G
    perm = rope_perm(HD)
    cosT, sinT = rope_tables(S, HD)
    csT = np.concatenate([cosT, sinT], axis=0)  # [HD, S]
    masks = causal_masks(TC)

    # Per-batch xT (shared across head shards)
    xt = {}
    for b in range(B):
        xt[b] = np.ascontiguousarray(x[b].T).astype(BF16)  # [D, S]

    # Per-headgroup weight shards
    wshard = {}
    for g in range(G):
        qrows = slice(g * NQ * HD, (g + 1) * NQ * HD)
        kvrows = slice(g * NKV * HD, (g + 1) * NKV * HD)
        wq_g = wq[qrows, :].copy()      # [NQ*HD, D]
        wk_g = wk[kvrows, :].copy()
        wv_g = wv[kvrows, :].copy()
        # RoPE permutation of output rows, per head
        for hh in range(NQ):
            blk = wq_g[hh * HD:(hh + 1) * HD]
            wq_g[hh * HD:(hh + 1) * HD] = blk[perm]
        for hh in range(NKV):
            blk = wk_g[hh * HD:(hh + 1) * HD]
            wk_g[hh * HD:(hh + 1) * HD] = blk[perm]
        wshard[g] = (
            np.ascontiguousarray(wq_g.T).astype(BF16),   # [D, NQ*HD]
            np.ascontiguousarray(wk_g.T).astype(BF16),
            np.ascontiguousarray(wv_g.T).astype(BF16),
            np.ascontiguousarray(wo[:, qrows].T).astype(BF16),  # [NQ*HD, D]
        )

    in_maps = []
    for b in range(n_batch_shards):
        for g in range(G):
            wqp, wkp, wvp, wot = wshard[g]
            in_maps.append({
                "xb": xt[b],
                "wqp": wqp, "wkp": wkp, "wvp": wvp, "wot": wot,
                "csT": csT,
                "masks": masks,
            })
    return in_maps


def combine_outputs(outTs, B, G):
    """outTs: list of [D, S] partials, core order b*G+g. Returns [B, S, D]."""
    outs = []
    for b in range(B):
        acc = outTs[b * G].astype(np.float32).copy()
        for g in range(1, G):
            acc += outTs[b * G + g]
        outs.append(acc.T)  # [S, D]
    return np.stack(outs)


_NC_CACHE = {}


def _get_nc(S, D, NQ, NKV, HD, TC):
    key = (S, D, NQ, NKV, HD, TC)
    if key not in _NC_CACHE:
        nc = build_attention_nc(S=S, D=D, NQ=NQ, NKV=NKV, HD=HD, TC=TC)
        split_multiwait_insts(nc)
        _NC_CACHE[key] = nc
    return _NC_CACHE[key]


def kernel(**inputs):
    x = np.asarray(inputs["x"], dtype=np.float32)
    wq = np.asarray(inputs["wq"], dtype=np.float32)
    wk = np.asarray(inputs["wk"], dtype=np.float32)
    wv = np.asarray(inputs["wv"], dtype=np.float32)
    wo = np.asarray(inputs["wo"], dtype=np.float32)

    B, S, D = x.shape          # (2, 2048, 2048)
    NQ_TOT = wq.shape[0] // 128
    NKV_TOT = wk.shape[0] // 128
    HD = 128
    TC = 512
    G = 4                      # head shards
    NQ, NKV = NQ_TOT // G, NKV_TOT // G

    nc = _get_nc(S, D, NQ, NKV, HD, TC)
    in_maps = make_in_maps(
        x, wq, wk, wv, wo,
        n_batch_shards=B, n_head_shards=G,
        NQ_TOT=NQ_TOT, NKV_TOT=NKV_TOT, HD=HD, TC=TC,
    )

    from concourse.bass_utils import run_bass_kernel_spmd

    trace = os.environ.get("BASS_ATTN_TRACE") == "1"
    res = run_bass_kernel_spmd(nc, in_maps, list(range(len(in_maps))), trace=trace)
    kernel.last_results = res
    outTs = [r["outT"] for r in res.results]
    return combine_outputs(outTs, B, G).astype(np.float32)
